# revision 24
# baseline (speedup 1.0000x reference)
"""Trainium2 Bass kernel for nn_ExpandedSchedule (ODE schedule solver).

Math: per-step 6x6 transform A_t = I + M_t*dt with dt = 5e-6 splits into
a 3x3 block (beta,kappa,nu) + 2x2 block (alpha,lam); component 5 and the
g-MLP never reach the output and are dropped (exact).

Because dt is tiny, over a chain of L=196 steps:
  - interior prefixes are first order: P_l ~ I + C_l  (err ~1e-5, local)
  - chain totals are second order: T ~ I + S1 + S2    (err ~1e-9/chain)
f, r are smooth scalar MLP outputs, so the MLP is sampled only at chain
endpoints (2 nodes/chain, piecewise-linear, interp err ~1e-10); every
per-chain quantity (integrals of f, r and their products) is then an
exact closed-form linear map W of 15 node products - computed on the PE
as transpose -> W matmul -> transpose.

Global scan: 1024 chains laid out [128 partitions x 8 blocks], chain
g = q*128 + (127-p) so prefixes accumulate toward partition 0.  A
10-pass global Hillis-Steele (7 PE partition-shift passes + 3 free-dim
block-shift passes) gives every chain its global inclusive prefix; the
exclusive prefix (shift by one chain) is applied directly to s0.  No
collectives, no mid-kernel DMA round-trips, no carry select.

Validated against the exact reference in numpy: rel Frobenius 1.1e-6.
"""

import sys
for _p in ("/opt/trn_rl_repo", "/root/.axon_site/_ro/trn_rl_repo"):
    if _p not in sys.path:
        sys.path.insert(0, _p)

import numpy as np

import concourse.bass as bass
import concourse.mybir as mybir
import concourse.tile as tile
from concourse.bass_utils import run_bass_kernel_spmd

F32 = mybir.dt.float32
F32R = mybir.dt.float32r
AF = mybir.ActivationFunctionType
ALU = mybir.AluOpType

T = 200001
N = T - 1                    # 200000 steps
L = 196                      # steps per chain
G = 1024                     # global chains
Q = 8                        # blocks (one per core)
CH = 128                     # chains per block (partition dim)
NCORES = 8
PERC = CH * L                # 25088 steps per block
SAMP = 2 * G                 # 2048 MLP sample slots (2 nodes/chain)
MT = 2                       # MLP tiles
TT = 1024                    # samples per MLP tile
SHIFT_DS = (1, 2, 4, 8, 16, 32, 64)

# cpack column layout
C_C1 = 0              # 196: (s+1)*dt  (also the per-point D vector)
C_C2 = C_C1 + L       # 196: s(s+1)/(2L)*dt
C_QM = C_C2 + L       # 8: qmask
C_B3 = C_QM + Q       # 16: b3 tiled x8
C_IDZ = C_B3 + 16     # 13: zeros except row 127 = identity row
C_S0 = C_IDZ + 13     # 3: s0 broadcast
C_IDF = C_S0 + 3      # 13: identity row on ALL partitions
C_IDP = C_IDF + 13    # 7*13: idpads for flipped shifts
C_W = C_IDP + 7 * 13  # 104: W block-diag (rows 0:120)
CPW = C_W + 104


def _combine33f(nc, pool, A, B, out, tag):
    """out = A @ B on flattened 3x3 entry views [P, nb, 9] (row-major)."""
    P, nb = A.shape[0], A.shape[1]
    A4 = A.rearrange("p b (i k) -> p b i k", i=3)
    B4 = B.rearrange("p b (k j) -> p b k j", k=3)
    O4 = out.rearrange("p b (i j) -> p b i j", i=3)
    ts = [pool.tile([128, nb, 3, 3], F32, tag=f"c33_{tag}_{i}",
                    name=f"c33_{tag}_{i}") for i in range(3)]
    for k in range(3):
        ak = A4[:, :, :, k].unsqueeze(3).broadcast_to([P, nb, 3, 3])
        bk = B4[:, :, k, :].unsqueeze(2).broadcast_to([P, nb, 3, 3])
        nc.vector.tensor_mul(out=ts[k][:P], in0=ak, in1=bk)
    nc.vector.tensor_add(out=ts[0][:P], in0=ts[0][:P], in1=ts[1][:P])
    nc.vector.tensor_add(out=O4, in0=ts[0][:P], in1=ts[2][:P])


def _combine22(nc, pool, A, B, out, tag):
    """out = A @ B on [P, nb, 4] views (Pool engine, 3 classic ops)."""
    P, nb = A.shape[0], A.shape[1]
    A4 = A.rearrange("p b (i k) -> p b i k", i=2)
    B4 = B.rearrange("p b (k j) -> p b k j", k=2)
    O4 = out.rearrange("p b (i j) -> p b i j", i=2)
    ts = [pool.tile([128, nb, 2, 2], F32, tag=f"c22_{tag}_{i}",
                    name=f"c22_{tag}_{i}") for i in range(2)]
    for k in range(2):
        ak = A4[:, :, :, k].unsqueeze(3).broadcast_to([P, nb, 2, 2])
        bk = B4[:, :, k, :].unsqueeze(2).broadcast_to([P, nb, 2, 2])
        nc.gpsimd.tensor_mul(out=ts[k][:P], in0=ak, in1=bk)
    nc.gpsimd.tensor_add(out=O4, in0=ts[0][:P], in1=ts[1][:P])


def _hoist_matmul_waits(nc):
    """Walrus codegen allows one sync wait per engine instruction; move
    extra waits onto inserted same-engine NoOps just before it."""
    for fn in nc.m.functions:
        for bb in fn.blocks:
            new = []
            for ins in bb.instructions:
                si = getattr(ins, "sync_info", None)
                if (si is not None and si.on_wait and len(si.on_wait) > 1
                        and getattr(ins, "engine", None) is not None):
                    waits = list(si.on_wait)
                    si.on_wait = [waits.pop()]
                    for wi, w in enumerate(waits):
                        new.append(mybir.InstNoOp(
                            name=f"{ins.name}-wgate{wi}", engine=ins.engine,
                            ins=[], outs=[],
                            sync_info=mybir.SyncInfo(on_wait=[w],
                                                     on_update=[])))
                new.append(ins)
            bb.instructions = new


def build_program(hoist=True, sim_safe=False):
    nc = bass.Bass()
    gelu_fn = AF.Relu if sim_safe else AF.Gelu

    tsf_d = nc.declare_dram_parameter("tsflat", [1, SAMP + 7 * 128], F32,
                                      isOutput=False)
    wp_d = nc.declare_dram_parameter("wpack", [128, 6], F32, isOutput=False)
    w2_d = nc.declare_dram_parameter("w2p", [128, 512], F32, isOutput=False)
    w3_d = nc.declare_dram_parameter("w3p", [128, 4], F32, isOutput=False)
    cp_d = nc.declare_dram_parameter("cpack", [128, CPW], F32,
                                     isOutput=False)
    sh_d = nc.declare_dram_parameter("shifts", [128, 15 * 128], F32,
                                     isOutput=False)
    out_d = nc.declare_dram_parameter("out", [CH, L * 7], F32, isOutput=True)

    with tile.TileContext(nc) as tc:
        with (
            tc.tile_pool(name="const", bufs=1) as cp,
            tc.tile_pool(name="main", bufs=1) as mp,
        ):
            # ---- constants to SBUF (MLP inputs first) ----
            tfl = cp.tile([1, SAMP + 7 * 128], F32)
            nc.sync.dma_start(out=tfl[:, :], in_=tsf_d[:, :])
            wsb = cp.tile([128, 6], F32)
            nc.sync.dma_start(out=wsb[:, :], in_=wp_d[:, :])
            w2sb = cp.tile([128, 512], F32R)
            nc.sync.dma_start(out=w2sb[:, :], in_=w2_d[:, :].bitcast(F32R))
            w3sb = cp.tile([128, 4], F32R)
            nc.sync.dma_start(out=w3sb[:, :], in_=w3_d[:, :].bitcast(F32R))
            csb = cp.tile([128, CPW], F32)
            nc.sync.dma_start(out=csb[:, :], in_=cp_d[:, :])
            shsb = cp.tile([128, 15 * 128], F32)
            nc.sync.dma_start(out=shsb[:, :], in_=sh_d[:, :])

            b1sl = wsb[:, 0:2]
            b2sl = wsb[:, 2:4]
            w1sl = wsb[:, 4:6]
            c1v = csb[:, C_C1:C_C1 + L]
            c2v = csb[:, C_C2:C_C2 + L]
            qmv = csb[:, C_QM:C_QM + Q]
            b3v = csb[:, C_B3:C_B3 + 16]
            idZ = csb[:, C_IDZ:C_IDZ + 13]
            s0v = csb[:, C_S0:C_S0 + 3]
            idF = csb[:, C_IDF:C_IDF + 13]

            def idp(di):
                return csb[:, C_IDP + 13 * di:C_IDP + 13 * (di + 1)]

            idn = shsb[:, 7 * 128:8 * 128]

            onesf = cp.tile([1, 128], F32)
            nc.vector.memset(onesf[:, :], 1.0)
            lnw = cp.tile([1, 1], F32)
            nc.vector.memset(lnw[:, :], 1.0)
            # trigger the gelu activation-table load before the DMAs land
            nc.scalar.activation(out=lnw[:, :], in_=lnw[:, :], func=gelu_fn,
                                 bias=0.0, scale=1.0)

            frs = mp.tile([128, 2 * 2 * Q], F32)   # col = (q*2+k)*2 + c

            # ---- phase 1: sampled fr-MLP ----
            with (
                tc.tile_pool(name="h1", bufs=2) as h1p,
                tc.tile_pool(name="h2", bufs=2) as h2p,
                tc.tile_pool(name="psB", bufs=1, space="PSUM") as psB,
                tc.tile_pool(name="ps2", bufs=2, space="PSUM") as ps2,
                tc.tile_pool(name="ps3", bufs=2, space="PSUM") as ps3,
            ):
                for ti in range(MT):
                    psb = psB.tile([128, TT], F32, tag="psb")
                    if ti == 0:
                        # PE p-state warm-up: junk broadcasts, overwritten
                        for _ in range(3):
                            nc.tensor.matmul(out=psb[:, 0:512],
                                             lhsT=onesf[:, :],
                                             rhs=tfl[0:1, 0:512],
                                             start=True, stop=True)
                    for hh in range(TT // 512):
                        nc.tensor.matmul(
                            out=psb[:, hh * 512:(hh + 1) * 512],
                            lhsT=onesf[:, :],
                            rhs=tfl[0:1, ti * TT + hh * 512:
                                    ti * TT + (hh + 1) * 512],
                            start=True, stop=True)
                    h1 = []
                    for mi in range(2):
                        h = h1p.tile([128, TT], F32R, tag=f"h1_{mi}")
                        nc.scalar.activation(out=h[:, :], in_=psb[:, :],
                                             func=gelu_fn,
                                             bias=b1sl[:, mi:mi + 1],
                                             scale=w1sl[:, mi:mi + 1])
                        h1.append(h)
                    h2 = []
                    for mi in range(2):
                        p2 = ps2.tile([128, TT], F32, tag="p2")
                        for hh in range(TT // 512):
                            for kt in range(2):
                                lhs = w2sb[:, kt * 256 + mi * 128:
                                           kt * 256 + (mi + 1) * 128]
                                nc.tensor.matmul(
                                    out=p2[:, hh * 512:(hh + 1) * 512],
                                    lhsT=lhs,
                                    rhs=h1[kt][:, hh * 512:(hh + 1) * 512],
                                    start=(kt == 0), stop=(kt == 1))
                        h = h2p.tile([128, TT], F32R, tag=f"h2_{mi}")
                        nc.scalar.activation(out=h[:, :], in_=p2[:, :],
                                             func=gelu_fn,
                                             bias=b2sl[:, mi:mi + 1],
                                             scale=1.0)
                        h2.append(h)
                    p3 = ps3.tile([128, 16], F32, tag="p3")
                    for j in range(TT // 128):
                        for kt in range(2):
                            nc.tensor.matmul(
                                out=p3[:, 2 * j:2 * j + 2],
                                lhsT=h2[kt][:, j * 128:(j + 1) * 128],
                                rhs=w3sb[:, 2 * kt:2 * kt + 2],
                                start=(kt == 0), stop=(kt == 1))
                    nc.vector.tensor_add(out=frs[:, 16 * ti:16 * ti + 16],
                                         in0=p3[:, :], in1=b3v)
                # preload the Ln activation table off the critical path
                nc.scalar.activation(out=lnw[:, :], in_=lnw[:, :],
                                     func=AF.Ln, bias=0.0, scale=1.0)

            # ---- phases 2+ ----
            with (
                tc.tile_pool(name="ip", bufs=1) as ip,
                tc.tile_pool(name="pp", bufs=1) as pp,
                tc.tile_pool(name="sc3", bufs=2) as sc3,
                tc.tile_pool(name="sc2", bufs=1) as sc2,
                tc.tile_pool(name="lvb", bufs=2) as lvb,
                tc.tile_pool(name="psR", bufs=1, space="PSUM") as psR,
                tc.tile_pool(name="psS", bufs=2, space="PSUM") as psS,
                tc.tile_pool(name="sm", bufs=2) as sm,
                tc.tile_pool(name="st", bufs=1) as stp,
            ):
                frsv = frs.rearrange("p (q k c) -> p q k c", q=Q, k=2)
                fr0q = frsv[:, :, 0, :]            # [128,8,2] (f0,r0)
                fr1q = frsv[:, :, 1, :]
                f0q = frsv[:, :, 0, 0]             # [128,8]
                f1q = frsv[:, :, 1, 0]
                nodecv = frs.rearrange("p (q k c) -> p q c k", q=Q, k=2)
                rpair = nodecv[:, :, 1, :]         # [128,8,2] (r0,r1)

                # (a) own-block node extraction (DVE)
                mkq = ip.tile([128, 32], F32, tag="mkq")
                mkqv = mkq.rearrange("p (k c q) -> p k c q", k=2, c=2)
                nc.vector.tensor_mul(
                    out=mkqv,
                    in0=frs.rearrange("p (q k c) -> p k c q", q=Q, k=2),
                    in1=qmv.unsqueeze(1).unsqueeze(1)
                    .broadcast_to([128, 2, 2, Q]))
                frown = ip.tile([128, 4], F32, tag="frown")
                nc.vector.tensor_reduce(out=frown.rearrange(
                    "p (k c) -> p k c", k=2), in_=mkqv,
                    axis=mybir.AxisListType.X, op=ALU.add)

                # (b) per-point F, R build (Pool)
                dfr = pp.tile([128, 2], F32, tag="dfr")
                nc.gpsimd.tensor_sub(out=dfr[:, :], in0=frown[:, 2:4],
                                     in1=frown[:, 0:2])
                FRpt = pp.tile([128, 2 * L], F32, tag="FRpt")
                tB = pp.tile([128, 2 * L], F32, tag="tB")
                FRv = FRpt.rearrange("p (c s) -> p c s", c=2)
                tBv = tB.rearrange("p (c s) -> p c s", c=2)
                nc.gpsimd.tensor_mul(
                    out=FRv, in0=frown[:, 0:2].unsqueeze(2)
                    .broadcast_to([128, 2, L]),
                    in1=c1v.unsqueeze(1).broadcast_to([128, 2, L]))
                nc.gpsimd.tensor_mul(
                    out=tBv, in0=dfr[:, :].unsqueeze(2)
                    .broadcast_to([128, 2, L]),
                    in1=c2v.unsqueeze(1).broadcast_to([128, 2, L]))
                nc.gpsimd.tensor_add(out=FRpt[:, :], in0=FRpt[:, :],
                                     in1=tB[:, :])
                Fpt = FRpt[:, 0:L]
                Rpt = FRpt[:, L:2 * L]

                # (c) IN tile: [1,f0,f1,r0,r1,ff00,ff01,ff11,rr00,rr01,
                #               rr11,fr00,fr01,fr10,fr11] per q
                IN = ip.tile([128, Q * 15], F32, tag="IN")
                INv = IN.rearrange("p (q s) -> p q s", q=Q)
                nc.gpsimd.tensor_copy(out=INv[:, :, 0],
                                      in_=idF[:, 0:1].broadcast_to([128, Q]))
                nc.vector.tensor_copy(out=INv[:, :, 1:5], in_=nodecv)
                nc.vector.tensor_mul(out=INv[:, :, 5:9:3], in0=fr0q,
                                     in1=fr0q)
                nc.vector.tensor_mul(out=INv[:, :, 6:10:3], in0=fr0q,
                                     in1=fr1q)
                nc.vector.tensor_mul(out=INv[:, :, 7:11:3], in0=fr1q,
                                     in1=fr1q)
                nc.gpsimd.tensor_mul(
                    out=INv[:, :, 11:13],
                    in0=f0q.unsqueeze(2).broadcast_to([128, Q, 2]),
                    in1=rpair)
                nc.gpsimd.tensor_mul(
                    out=INv[:, :, 13:15],
                    in0=f1q.unsqueeze(2).broadcast_to([128, Q, 2]),
                    in1=rpair)

                # (d) chain totals T = v @ W via PE transpose sandwich
                psT = psR.tile([120, 128], F32, tag="psT")
                nc.tensor.transpose(out=psT[:, :], in_=IN[:, :],
                                    identity=idn)
                b1t = sm.tile([120, 128], F32, tag="b1t")
                nc.vector.tensor_copy(out=b1t[:, :], in_=psT[:, :])
                psA = psR.tile([104, 128], F32, tag="psA")
                nc.tensor.matmul(out=psA[:, :],
                                 lhsT=csb[0:120, C_W:C_W + 104],
                                 rhs=b1t[:, :], start=True, stop=True)
                b2t = sm.tile([104, 128], F32, tag="b2t")
                nc.vector.tensor_copy(out=b2t[:, :], in_=psA[:, :])
                psT2 = psR.tile([128, 104], F32, tag="psT2")
                nc.tensor.transpose(out=psT2[:, :], in_=b2t[:, :],
                                    identity=idn[0:104, 0:104])
                Trow = lvb.tile([128, Q * 13], F32, tag="T")
                nc.vector.tensor_copy(out=Trow[:, :], in_=psT2[:, :])

                # (e) global Hillis-Steele: 7 partition passes with
                # cross-block wrap (pull overflow from previous block)
                Tcur = Trow
                for di, d in enumerate(SHIFT_DS):
                    pr = psS.tile([128, Q * 13], F32, tag="pr")
                    nc.tensor.matmul(out=pr[:, :],
                                     lhsT=shsb[:, di * 128:(di + 1) * 128],
                                     rhs=Tcur[:, :], start=True, stop=False)
                    nc.tensor.matmul(out=pr[:, 13:Q * 13],
                                     lhsT=shsb[:, (8 + di) * 128:
                                               (9 + di) * 128],
                                     rhs=Tcur[:, 0:(Q - 1) * 13],
                                     start=False, stop=False,
                                     skip_group_check=True)
                    # identity fill for global out-of-range rows (q=0)
                    nc.tensor.matmul(out=pr[:, 0:13],
                                     lhsT=tfl[0:1, SAMP + di * 128:
                                              SAMP + (di + 1) * 128],
                                     rhs=idF[0:1, :],
                                     start=False, stop=True,
                                     skip_group_check=True)
                    Tn = lvb.tile([128, Q * 13], F32, tag="T")
                    TcV = Tcur.rearrange("p (q e) -> p q e", q=Q)
                    TnV = Tn.rearrange("p (q e) -> p q e", q=Q)
                    prV = pr.rearrange("p (q e) -> p q e", q=Q)
                    # Pool cannot read PSUM: stage the 2x2 columns to SBUF
                    B22 = sm.tile([128, Q * 4], F32, tag="B22")
                    B22v = B22.rearrange("p (q e) -> p q e", q=Q)
                    nc.vector.tensor_copy(out=B22v, in_=prV[:, :, 9:13])
                    _combine33f(nc, sc3, TcV[:, :, 0:9], prV[:, :, 0:9],
                                TnV[:, :, 0:9], "e")
                    _combine22(nc, sc2, TcV[:, :, 9:13], B22v,
                               TnV[:, :, 9:13], "e")
                    Tcur = Tn

                # (f) 3 free-dim block passes (shift by d blocks)
                for d in (1, 2, 4):
                    Tn = lvb.tile([128, Q * 13], F32, tag="T")
                    TcV = Tcur.rearrange("p (q e) -> p q e", q=Q)
                    TnV = Tn.rearrange("p (q e) -> p q e", q=Q)
                    nc.vector.tensor_copy(out=TnV[:, 0:d, :],
                                          in_=TcV[:, 0:d, :])
                    _combine33f(nc, sc3, TcV[:, d:Q, 0:9],
                                TcV[:, 0:Q - d, 0:9],
                                TnV[:, d:Q, 0:9], "f")
                    _combine22(nc, sc2, TcV[:, d:Q, 9:13],
                               TcV[:, 0:Q - d, 9:13],
                               TnV[:, d:Q, 9:13], "f")
                    Tcur = Tn

                # (g) global exclusive prefix -> per-chain x
                prx = psR.tile([128, Q * 13], F32, tag="prx")
                nc.tensor.matmul(out=prx[:, :], lhsT=shsb[:, 0:128],
                                 rhs=Tcur[:, :], start=True, stop=False)
                # row 127 (chain j=0 of block q) += block q-1 total, via a
                # partition-0 -> 127 routing matrix on the q-shifted view
                nc.tensor.matmul(out=prx[:, 13:Q * 13],
                                 lhsT=shsb[:, 8 * 128:9 * 128],
                                 rhs=Tcur[:, 0:(Q - 1) * 13],
                                 start=False, stop=True,
                                 skip_group_check=True)
                Rexc = sm.tile([128, Q * 13], F32, tag="Rexc")
                RxV = Rexc.rearrange("p (q e) -> p q e", q=Q)
                prV = prx.rearrange("p (q e) -> p q e", q=Q)
                nc.vector.tensor_copy(out=RxV[:, 1:Q, :], in_=prV[:, 1:Q, :])
                nc.vector.tensor_add(out=RxV[:, 0, :], in0=prV[:, 0, :],
                                     in1=idZ)

                # own-block rows
                tq = sm.tile([128, 13 * Q], F32, tag="tq")
                tqV = tq.rearrange("p (e q) -> p e q", e=13)
                nc.vector.tensor_mul(
                    out=tqV, in0=Rexc.rearrange("p (q e) -> p e q", q=Q),
                    in1=qmv.unsqueeze(1).broadcast_to([128, 13, Q]))
                Rown = sm.tile([128, 13], F32, tag="Rown")
                nc.vector.tensor_reduce(out=Rown[:, :], in_=tqV,
                                        axis=mybir.AxisListType.X,
                                        op=ALU.add)
                Rx3 = Rown[:, 0:9].rearrange("p (i j) -> p i j", i=3)
                x3 = sm.tile([128, 3], F32, tag="x3")
                nc.vector.tensor_scalar(out=x3[:, :], in0=Rx3[:, :, 0],
                                        scalar1=s0v[:, 0:1], scalar2=None,
                                        op0=ALU.mult)
                nc.vector.scalar_tensor_tensor(out=x3[:, :],
                                               in0=Rx3[:, :, 1],
                                               scalar=s0v[:, 1:2],
                                               in1=x3[:, :], op0=ALU.mult,
                                               op1=ALU.add)
                nc.vector.scalar_tensor_tensor(out=x3[:, :],
                                               in0=Rx3[:, :, 2],
                                               scalar=s0v[:, 2:3],
                                               in1=x3[:, :], op0=ALU.mult,
                                               op1=ALU.add)
                X0 = x3[:, 0:1]
                X1 = x3[:, 1:2]
                X2 = x3[:, 2:3]
                X3c = Rown[:, 9:10]     # alpha carry = 2x2 col0 row0
                X4 = Rown[:, 11:12]     # lam carry  = 2x2 col0 row1

                # (h) states -> o7 strided columns
                out7 = stp.tile([CH, L * 7], F32, tag="out7")
                o7 = out7.rearrange("p (l c) -> p l c", c=7)
                p2x0 = sm.tile([CH, 1], F32, tag="p2x0")
                nx1 = sm.tile([CH, 1], F32, tag="nx1")
                n2x2 = sm.tile([CH, 1], F32, tag="n2x2")
                nx4 = sm.tile([CH, 1], F32, tag="nx4")
                nc.vector.tensor_scalar(out=p2x0[:, :], in0=X0, scalar1=2.0,
                                        scalar2=None, op0=ALU.mult)
                nc.vector.tensor_scalar(out=nx1[:, :], in0=X1, scalar1=-1.0,
                                        scalar2=None, op0=ALU.mult)
                nc.vector.tensor_scalar(out=n2x2[:, :], in0=X2, scalar1=-2.0,
                                        scalar2=None, op0=ALU.mult)
                nc.vector.tensor_scalar(out=nx4[:, :], in0=X4, scalar1=-1.0,
                                        scalar2=None, op0=ALU.mult)
                # DVE: beta, kappa, nu
                nc.vector.tensor_scalar(out=o7[:, :, 2], in0=Rpt,
                                        scalar1=nx1[:, 0:1], scalar2=X0,
                                        op0=ALU.mult, op1=ALU.add)
                tk = stp.tile([CH, L], F32, tag="tk")
                nc.vector.tensor_scalar(out=tk[:, :], in0=c1v,
                                        scalar1=p2x0[:, 0:1], scalar2=X1,
                                        op0=ALU.mult, op1=ALU.add)
                nc.vector.scalar_tensor_tensor(out=tk[:, :], in0=Fpt,
                                               scalar=nx1[:, 0:1],
                                               in1=tk[:, :], op0=ALU.mult,
                                               op1=ALU.add)
                nc.vector.scalar_tensor_tensor(out=o7[:, :, 3], in0=Rpt,
                                               scalar=n2x2[:, 0:1],
                                               in1=tk[:, :], op0=ALU.mult,
                                               op1=ALU.add)
                tn_ = stp.tile([CH, L], F32, tag="tn_")
                nc.vector.tensor_scalar(out=tn_[:, :], in0=c1v,
                                        scalar1=X1, scalar2=X2,
                                        op0=ALU.mult, op1=ALU.add)
                nc.vector.scalar_tensor_tensor(out=o7[:, :, 5], in0=Fpt,
                                               scalar=n2x2[:, 0:1],
                                               in1=tn_[:, :], op0=ALU.mult,
                                               op1=ALU.add)
                # Pool: alpha, lam
                tm_ = stp.tile([CH, L], F32, tag="tm_")
                tl_ = stp.tile([CH, L], F32, tag="tl_")
                nc.gpsimd.tensor_mul(out=tm_[:, :], in0=Rpt,
                                     in1=nx4[:, 0:1].broadcast_to([CH, L]))
                nc.gpsimd.tensor_add(out=o7[:, :, 0], in0=tm_[:, :],
                                     in1=X3c.broadcast_to([CH, L]))
                nc.gpsimd.tensor_mul(out=tl_[:, :], in0=c1v,
                                     in1=X3c.broadcast_to([CH, L]))
                nc.gpsimd.tensor_add(out=tl_[:, :], in0=tl_[:, :],
                                     in1=X4.broadcast_to([CH, L]))
                nc.gpsimd.tensor_mul(out=tm_[:, :], in0=Fpt,
                                     in1=nx4[:, 0:1].broadcast_to([CH, L]))
                nc.gpsimd.tensor_add(out=o7[:, :, 1], in0=tl_[:, :],
                                     in1=tm_[:, :])
                # kappa duplicate (cov is symmetric)
                nc.vector.tensor_copy(out=o7[:, :, 4], in_=o7[:, :, 3])

                # (i) log-SNR
                alp = o7[:, :, 0]
                lam = o7[:, :, 1]
                beta = o7[:, :, 2]
                kap = o7[:, :, 3]
                nu = o7[:, :, 5]
                ta = stp.tile([CH, L], F32, tag="ta")
                tb2 = stp.tile([CH, L], F32, tag="tb2")
                tcx = stp.tile([CH, L], F32, tag="tcx")
                td = stp.tile([CH, L], F32, tag="td")
                nc.vector.tensor_mul(out=ta[:, :], in0=lam, in1=lam)
                nc.vector.tensor_mul(out=ta[:, :], in0=beta, in1=ta[:, :])
                nc.vector.tensor_mul(out=tb2[:, :], in0=alp, in1=alp)
                nc.vector.tensor_mul(out=tb2[:, :], in0=nu, in1=tb2[:, :])
                nc.vector.tensor_add(out=ta[:, :], in0=ta[:, :],
                                     in1=tb2[:, :])
                nc.vector.tensor_mul(out=tb2[:, :], in0=alp, in1=lam)
                nc.vector.tensor_mul(out=tb2[:, :], in0=kap, in1=tb2[:, :])
                nc.vector.scalar_tensor_tensor(out=ta[:, :], in0=tb2[:, :],
                                               scalar=-2.0, in1=ta[:, :],
                                               op0=ALU.mult, op1=ALU.add)
                nc.gpsimd.tensor_mul(out=tcx[:, :], in0=kap, in1=kap)
                nc.gpsimd.tensor_mul(out=td[:, :], in0=beta, in1=nu)
                nc.gpsimd.tensor_sub(out=td[:, :], in0=td[:, :],
                                     in1=tcx[:, :])
                nc.scalar.activation(out=ta[:, :], in_=ta[:, :], func=AF.Ln,
                                     bias=0.0, scale=1.0)
                nc.scalar.activation(out=td[:, :], in_=td[:, :], func=AF.Ln,
                                     bias=0.0, scale=1.0)
                nc.vector.tensor_sub(out=o7[:, :, 6], in0=ta[:, :],
                                     in1=td[:, :])

                nc.sync.dma_start(out=out_d[:, :], in_=out7[:, :])
    if hoist:
        _hoist_matmul_waits(nc)
    return nc


_NC_CACHE = None
TRACE = False
LAST_EXEC_NS = None


def _w_matrix(dt):
    """Exact 13-entry chain-total map of the 15 node products."""
    A, B, C = 3777475 / 784, 3751865 / 392, 3701035 / 784
    D, E = 1242085 / 784, 6261645 / 784
    d2 = dt * dt
    FLf = np.array([98.5, 97.5]) * dt
    IfDc = np.array([6402.5, 12707.5]) * d2
    IFc = np.array([12805.0, 6305.0]) * d2
    ID = 19110 * d2
    DL = L * dt
    ffF = np.array([A, B, C]) * d2
    fRx = np.array([A, D, E, C]) * d2
    rFx = np.array([A, E, D, C]) * d2
    W = np.zeros((15, 13), np.float64)
    W[0, 0] = 1.0
    W[3, 0], W[4, 0] = -2 * IfDc
    W[3, 1], W[4, 1] = -FLf
    W[11:15, 1] = rFx
    W[8:11, 2] = 2 * ffF
    W[0, 3] = 2 * DL
    W[1, 3], W[2, 3] = -2 * IfDc
    W[0, 4] = 1.0
    W[1, 4], W[2, 4] = -FLf
    W[3, 4] = -2 * IFc[0] - 2 * IfDc[0]
    W[4, 4] = -2 * IFc[1] - 2 * IfDc[1]
    W[5:8, 4] = ffF
    W[3, 5], W[4, 5] = -2 * FLf
    W[11:15, 5] = 2 * fRx + 4 * rFx
    W[0, 6] = 2 * ID
    W[0, 7] = DL
    W[1, 7] = -IFc[0] - 2 * IfDc[0]
    W[2, 7] = -IFc[1] - 2 * IfDc[1]
    W[0, 8] = 1.0
    W[1, 8], W[2, 8] = -2 * FLf
    W[3, 8], W[4, 8] = -2 * IFc
    W[5:8, 8] = 4 * ffF
    W[0, 9] = 1.0
    W[3, 9], W[4, 9] = -IfDc
    W[:, 10] = W[:, 1]
    W[0, 11] = DL
    W[1, 11], W[2, 11] = -IfDc
    W[0, 12] = 1.0
    W[1, 12] = -FLf[0] - IFc[0]
    W[2, 12] = -FLf[1] - IFc[1]
    W[5:8, 12] = ffF
    return W


def kernel(**inputs):
    global _NC_CACHE, LAST_EXEC_NS
    t = np.asarray(inputs["t_range"], np.float32)
    t64 = t.astype(np.float64)
    dt = float((t64[-1] - t64[0]) / N)

    def f32(x):
        return np.ascontiguousarray(np.asarray(x, np.float32))

    w1cat = f32(inputs["fr_W1"])[:, 0]
    b1cat = f32(inputs["fr_b1"])
    w2t = f32(inputs["fr_W2"]).T            # [k, j]
    b2cat = f32(inputs["fr_b2"])
    w3t = f32(inputs["fr_W3"]).T            # [k, 2]
    b3 = f32(inputs["fr_b3"])

    lbn = f32(inputs["log_beta_nu_zero"])
    beta0 = np.float32(np.exp(lbn[0]))
    nu0 = np.float32(np.exp(lbn[1]))
    rho0 = np.float32(1.0 / (1.0 + np.exp(-f32(inputs["log_rho_zero"])[0])))
    kappa0 = np.float32(rho0 * np.sqrt(beta0) * np.sqrt(nu0))

    wpack = np.zeros((128, 6), np.float32)
    wpack[:, 0] = b1cat[0:128]
    wpack[:, 1] = b1cat[128:256]
    wpack[:, 2] = b2cat[0:128]
    wpack[:, 3] = b2cat[128:256]
    wpack[:, 4] = w1cat[0:128]
    wpack[:, 5] = w1cat[128:256]
    w2p = np.zeros((128, 512), np.float32)
    w2p[:, 0:256] = w2t[0:128, :]
    w2p[:, 256:512] = w2t[128:256, :]
    w3p = np.zeros((128, 4), np.float32)
    w3p[:, 0:2] = w3t[0:128, :]
    w3p[:, 2:4] = w3t[128:256, :]

    # sample nodes, flipped chain<->partition map: chain g = q*128+(127-p)
    p_arr = np.arange(128)
    tsflat = np.zeros((1, SAMP + 7 * 128), np.float32)
    for di, d in enumerate(SHIFT_DS):
        tsflat[0, SAMP + di * 128 + (128 - d):SAMP + (di + 1) * 128] = 1.0
    for q in range(Q):
        for k in range(2):
            gi = q * 128 + (127 - p_arr)
            idxs = np.minimum(gi * L + L * k, N)
            tsflat[0, (q * 2 + k) * 128 + p_arr] = t[idxs]

    s_arr = np.arange(L, dtype=np.float64)
    idrow = np.array([1, 0, 0, 0, 1, 0, 0, 0, 1, 1, 0, 0, 1], np.float32)
    cpack = np.zeros((128, CPW), np.float32)
    cpack[:, C_C1:C_C1 + L] = ((s_arr + 1.0) * dt)[None, :]
    cpack[:, C_C2:C_C2 + L] = (s_arr * (s_arr + 1.0) / (2.0 * L) * dt)[None, :]
    cpack[:, C_B3:C_B3 + 16] = np.tile(b3, 8)[None, :]
    cpack[127, C_IDZ:C_IDZ + 13] = idrow
    cpack[:, C_S0:C_S0 + 3] = np.array([beta0, kappa0, nu0],
                                       np.float32)[None, :]
    cpack[:, C_IDF:C_IDF + 13] = idrow[None, :]
    for di, d in enumerate(SHIFT_DS):
        cpack[128 - d:, C_IDP + 13 * di:C_IDP + 13 * (di + 1)] = idrow[None, :]
    Wm = _w_matrix(dt).astype(np.float32)
    for q in range(Q):
        cpack[q * 15:(q + 1) * 15, C_W + q * 13:C_W + (q + 1) * 13] = Wm

    shifts = np.zeros((128, 15 * 128), np.float32)
    for di, d in enumerate(SHIFT_DS):
        shifts[:, di * 128:(di + 1) * 128] = np.eye(128, k=-d,
                                                    dtype=np.float32)
    shifts[:, 7 * 128:8 * 128] = np.eye(128, dtype=np.float32)
    for di, d in enumerate(SHIFT_DS):
        shifts[:, (8 + di) * 128:(9 + di) * 128] = np.eye(
            128, k=128 - d, dtype=np.float32)

    in_maps = []
    for c in range(NCORES):
        cpk = cpack.copy()
        cpk[:, C_QM + c] = 1.0
        in_maps.append({
            "tsflat": tsflat, "wpack": wpack, "w2p": w2p, "w3p": w3p,
            "cpack": cpk, "shifts": shifts,
        })

    if _NC_CACHE is None:
        _NC_CACHE = build_program()
    nc = _NC_CACHE
    res = run_bass_kernel_spmd(nc, in_maps, core_ids=list(range(NCORES)),
                               trace=TRACE)
    LAST_EXEC_NS = res.exec_time_ns

    full = np.empty((T, 7), np.float32)
    lsnr0 = np.float32(np.log(nu0) - np.log(beta0 * nu0 - kappa0 ** 2))
    full[0] = [1.0, 0.0, beta0, kappa0, kappa0, nu0, lsnr0]
    for c in range(NCORES):
        o = np.asarray(res.results[c]["out"], np.float32)
        o = o[::-1, :].reshape(PERC, 7)        # un-flip partitions
        lo = c * PERC
        hi = min(lo + PERC, N)
        full[lo + 1:hi + 1] = o[:hi - lo]
    return full


# revision 25
# speedup vs baseline: 1.2210x; 1.2210x over previous
"""Trainium2 Bass kernel for nn_ExpandedSchedule (ODE schedule solver).

Math: per-step 6x6 transform A_t = I + M_t*dt with dt = 5e-6.  Component
5 and the g-MLP never reach the output (dropped, exact).  The remaining
5 components split into a 2x2 block (alpha,lam) and a 3x3 block
(beta,kappa,nu) - and the 3x3 block is the SYMMETRIC SQUARE of the 2x2
one (kappa = 2*Sigma01 of the covariance evolution), so only 2x2
matrices are ever scanned (4 floats/chain); the 3-vector carry is lifted
from the 2x2 global prefix with a handful of per-chain products.

Because dt is tiny, over a chain of L=196 steps interior prefixes are
first order (I + C_l) and chain totals are second order (I + S1 + S2,
err ~1e-9/chain).  f, r are smooth scalar MLP outputs: the MLP is
sampled at chain endpoints only (2048 points, piecewise-linear, interp
err ~1e-10), and each chain's 2x2 total is an exact linear map W of 12
node products, computed on the PE as transpose -> W matmul -> transpose.

Global scan: 1024 chains laid out [128 partitions x 8 blocks], chain
g = q*128 + (127-p).  A 10-pass global Hillis-Steele (7 PE
partition-shift passes with cross-block wrap + 3 free-dim block-shift
passes) gives every chain its global prefix locally on every core - no
collectives, no mid-kernel DMA round-trips.

Validated against the exact reference in numpy: rel Frobenius 1.1e-6.
"""

import sys
for _p in ("/opt/trn_rl_repo", "/root/.axon_site/_ro/trn_rl_repo"):
    if _p not in sys.path:
        sys.path.insert(0, _p)

import numpy as np

import concourse.bass as bass
import concourse.mybir as mybir
import concourse.tile as tile
from concourse.bass_utils import run_bass_kernel_spmd

F32 = mybir.dt.float32
F32R = mybir.dt.float32r
AF = mybir.ActivationFunctionType
ALU = mybir.AluOpType

T = 200001
N = T - 1                    # 200000 steps
L = 196                      # steps per chain
G = 1024                     # global chains
Q = 8                        # blocks (one per core)
CH = 128                     # chains per block (partition dim)
NCORES = 8
PERC = CH * L                # 25088 steps per block
SAMP = 2 * G                 # 2048 MLP sample slots (2 nodes/chain)
MT = 2                       # MLP tiles
TT = 1024                    # samples per MLP tile
SHIFT_DS = (1, 2, 4, 8, 16, 32, 64)

# cpack column layout
C_C1 = 0              # 196: (s+1)*dt  (also the per-point D vector)
C_C2 = C_C1 + L       # 196: s(s+1)/(2L)*dt
C_QM = C_C2 + L       # 8: qmask
C_B3 = C_QM + Q       # 16: b3 tiled x8
C_IDZ = C_B3 + 16     # 4: zeros except row 127 = 2x2 identity row
C_S0 = C_IDZ + 4      # 3: s0 = (beta0, kappa0, nu0) broadcast
C_IDF = C_S0 + 3      # 4: 2x2 identity row on ALL partitions
C_V = C_IDF + 4       # 7*128: idfill lhsT rows (row 0 only)
C_W = C_V + 7 * 128   # 32: W block-diag (rows 0:96)
CPW = C_W + 32


def _c22(nc, pool, A, B, out, tag):
    """out = A @ B on flattened 2x2 entry views [P, nb, 4] (DVE)."""
    P, nb = A.shape[0], A.shape[1]
    A4 = A.rearrange("p b (i k) -> p b i k", i=2)
    B4 = B.rearrange("p b (k j) -> p b k j", k=2)
    O4 = out.rearrange("p b (i j) -> p b i j", i=2)
    ts = [pool.tile([128, nb, 2, 2], F32, tag=f"c22_{tag}_{i}",
                    name=f"c22_{tag}_{i}") for i in range(2)]
    for k in range(2):
        ak = A4[:, :, :, k].unsqueeze(3).broadcast_to([P, nb, 2, 2])
        bk = B4[:, :, k, :].unsqueeze(2).broadcast_to([P, nb, 2, 2])
        nc.vector.tensor_mul(out=ts[k][:P], in0=ak, in1=bk)
    nc.vector.tensor_add(out=O4, in0=ts[0][:P], in1=ts[1][:P])


def _hoist_matmul_waits(nc):
    """Walrus codegen allows one sync wait per engine instruction; move
    extra waits onto inserted same-engine NoOps just before it."""
    for fn in nc.m.functions:
        for bb in fn.blocks:
            new = []
            for ins in bb.instructions:
                si = getattr(ins, "sync_info", None)
                if (si is not None and si.on_wait and len(si.on_wait) > 1
                        and getattr(ins, "engine", None) is not None):
                    waits = list(si.on_wait)
                    si.on_wait = [waits.pop()]
                    for wi, w in enumerate(waits):
                        new.append(mybir.InstNoOp(
                            name=f"{ins.name}-wgate{wi}", engine=ins.engine,
                            ins=[], outs=[],
                            sync_info=mybir.SyncInfo(on_wait=[w],
                                                     on_update=[])))
                new.append(ins)
            bb.instructions = new


def build_program(hoist=True, sim_safe=False):
    nc = bass.Bass()
    gelu_fn = AF.Relu if sim_safe else AF.Gelu

    tsf_d = nc.declare_dram_parameter("tsflat", [1, SAMP], F32,
                                      isOutput=False)
    wp_d = nc.declare_dram_parameter("wpack", [128, 6], F32, isOutput=False)
    w2_d = nc.declare_dram_parameter("w2p", [128, 512], F32, isOutput=False)
    w3_d = nc.declare_dram_parameter("w3p", [128, 4], F32, isOutput=False)
    cp_d = nc.declare_dram_parameter("cpack", [128, CPW], F32,
                                     isOutput=False)
    sh_d = nc.declare_dram_parameter("shifts", [128, 15 * 128], F32,
                                     isOutput=False)
    out_d = nc.declare_dram_parameter("out", [CH, L * 7], F32, isOutput=True)

    with tile.TileContext(nc) as tc:
        with (
            tc.tile_pool(name="const", bufs=1) as cp,
            tc.tile_pool(name="main", bufs=1) as mp,
        ):
            # ---- constants to SBUF (MLP inputs first) ----
            tfl = cp.tile([1, SAMP], F32R)
            nc.sync.dma_start(out=tfl[:, :], in_=tsf_d[:, :].bitcast(F32R))
            wsb = cp.tile([128, 6], F32)
            nc.sync.dma_start(out=wsb[:, :], in_=wp_d[:, :])
            w2sb = cp.tile([128, 512], F32R)
            nc.sync.dma_start(out=w2sb[:, :], in_=w2_d[:, :].bitcast(F32R))
            w3sb = cp.tile([128, 4], F32R)
            nc.sync.dma_start(out=w3sb[:, :], in_=w3_d[:, :].bitcast(F32R))
            csb = cp.tile([128, CPW], F32)
            nc.sync.dma_start(out=csb[:, :], in_=cp_d[:, :])
            shsb = cp.tile([128, 15 * 128], F32)
            nc.sync.dma_start(out=shsb[:, :], in_=sh_d[:, :])

            b1sl = wsb[:, 0:2]
            b2sl = wsb[:, 2:4]
            w1sl = wsb[:, 4:6]
            c1v = csb[:, C_C1:C_C1 + L]
            c2v = csb[:, C_C2:C_C2 + L]
            qmv = csb[:, C_QM:C_QM + Q]
            b3v = csb[:, C_B3:C_B3 + 16]
            idZ = csb[:, C_IDZ:C_IDZ + 4]
            s0v = csb[:, C_S0:C_S0 + 3]
            idF = csb[:, C_IDF:C_IDF + 4]
            idn = shsb[:, 7 * 128:8 * 128]

            onesf = cp.tile([1, 128], F32)
            nc.vector.memset(onesf[:, :], 1.0)
            onesb = cp.tile([1, 128], F32R)
            nc.scalar.copy(out=onesb[:, :], in_=onesf[:, :])
            lnw = cp.tile([1, 1], F32)
            nc.vector.memset(lnw[:, :], 1.0)

            frs = mp.tile([128, 2 * 2 * Q], F32)   # col = (q*2+k)*2 + c

            # ---- phase 1: sampled fr-MLP ----
            with (
                tc.tile_pool(name="h1", bufs=2) as h1p,
                tc.tile_pool(name="h2", bufs=2) as h2p,
                tc.tile_pool(name="psB", bufs=1, space="PSUM") as psB,
                tc.tile_pool(name="ps2", bufs=2, space="PSUM") as ps2,
                tc.tile_pool(name="ps3", bufs=2, space="PSUM") as ps3,
            ):
                for ti in range(MT):
                    psb = psB.tile([128, TT], F32, tag="psb")
                    if ti == 0:
                        # PE p-state warm-up: junk broadcast, overwritten
                        nc.tensor.matmul(out=psb[:, 0:512],
                                         lhsT=onesb[:, :],
                                         rhs=tfl[0:1, 0:512],
                                         start=True, stop=True)
                    for hh in range(TT // 512):
                        nc.tensor.matmul(
                            out=psb[:, hh * 512:(hh + 1) * 512],
                            lhsT=onesb[:, :],
                            rhs=tfl[0:1, ti * TT + hh * 512:
                                    ti * TT + (hh + 1) * 512],
                            start=True, stop=True)
                    h1 = []
                    for mi in range(2):
                        h = h1p.tile([128, TT], F32R, tag=f"h1_{mi}")
                        nc.scalar.activation(out=h[:, :], in_=psb[:, :],
                                             func=gelu_fn,
                                             bias=b1sl[:, mi:mi + 1],
                                             scale=w1sl[:, mi:mi + 1])
                        h1.append(h)
                    h2 = []
                    for mi in range(2):
                        p2 = ps2.tile([128, TT], F32, tag="p2")
                        for hh in range(TT // 512):
                            for kt in range(2):
                                lhs = w2sb[:, kt * 256 + mi * 128:
                                           kt * 256 + (mi + 1) * 128]
                                nc.tensor.matmul(
                                    out=p2[:, hh * 512:(hh + 1) * 512],
                                    lhsT=lhs,
                                    rhs=h1[kt][:, hh * 512:(hh + 1) * 512],
                                    start=(kt == 0), stop=(kt == 1))
                        h = h2p.tile([128, TT], F32R, tag=f"h2_{mi}")
                        nc.scalar.activation(out=h[:, :], in_=p2[:, :],
                                             func=gelu_fn,
                                             bias=b2sl[:, mi:mi + 1],
                                             scale=1.0)
                        h2.append(h)
                    p3 = ps3.tile([128, 16], F32, tag="p3")
                    for j in range(TT // 128):
                        for kt in range(2):
                            nc.tensor.matmul(
                                out=p3[:, 2 * j:2 * j + 2],
                                lhsT=h2[kt][:, j * 128:(j + 1) * 128],
                                rhs=w3sb[:, 2 * kt:2 * kt + 2],
                                start=(kt == 0), stop=(kt == 1))
                    nc.vector.tensor_add(out=frs[:, 16 * ti:16 * ti + 16],
                                         in0=p3[:, :], in1=b3v)
                # preload the Ln activation table off the critical path
                nc.scalar.activation(out=lnw[:, :], in_=lnw[:, :],
                                     func=AF.Ln, bias=0.0, scale=1.0)

            # ---- phases 2+ ----
            with (
                tc.tile_pool(name="ip", bufs=1) as ip,
                tc.tile_pool(name="pp", bufs=1) as pp,
                tc.tile_pool(name="sc2", bufs=2) as sc2,
                tc.tile_pool(name="lvb", bufs=2) as lvb,
                tc.tile_pool(name="psR", bufs=1, space="PSUM") as psR,
                tc.tile_pool(name="psS", bufs=2, space="PSUM") as psS,
                tc.tile_pool(name="sm", bufs=2) as sm,
                tc.tile_pool(name="st", bufs=1) as stp,
            ):
                frsv = frs.rearrange("p (q k c) -> p q k c", q=Q, k=2)
                f0q = frsv[:, :, 0, 0]             # [128,8]
                f1q = frsv[:, :, 1, 0]
                nodecv = frs.rearrange("p (q k c) -> p q c k", q=Q, k=2)
                rpair = nodecv[:, :, 1, :]         # [128,8,2] (r0,r1)

                # (a) own-block node extraction (DVE)
                mkq = ip.tile([128, 32], F32, tag="mkq")
                mkqv = mkq.rearrange("p (k c q) -> p k c q", k=2, c=2)
                nc.vector.tensor_mul(
                    out=mkqv,
                    in0=frs.rearrange("p (q k c) -> p k c q", q=Q, k=2),
                    in1=qmv.unsqueeze(1).unsqueeze(1)
                    .broadcast_to([128, 2, 2, Q]))
                frown = ip.tile([128, 4], F32, tag="frown")
                nc.vector.tensor_reduce(out=frown.rearrange(
                    "p (k c) -> p k c", k=2), in_=mkqv,
                    axis=mybir.AxisListType.X, op=ALU.add)

                # (b) per-point F, R build (Pool)
                dfr = pp.tile([128, 2], F32, tag="dfr")
                nc.gpsimd.tensor_sub(out=dfr[:, :], in0=frown[:, 2:4],
                                     in1=frown[:, 0:2])
                FRpt = pp.tile([128, 2 * L], F32, tag="FRpt")
                tB = pp.tile([128, 2 * L], F32, tag="tB")
                FRv = FRpt.rearrange("p (c s) -> p c s", c=2)
                tBv = tB.rearrange("p (c s) -> p c s", c=2)
                nc.gpsimd.tensor_mul(
                    out=FRv, in0=frown[:, 0:2].unsqueeze(2)
                    .broadcast_to([128, 2, L]),
                    in1=c1v.unsqueeze(1).broadcast_to([128, 2, L]))
                nc.gpsimd.tensor_mul(
                    out=tBv, in0=dfr[:, :].unsqueeze(2)
                    .broadcast_to([128, 2, L]),
                    in1=c2v.unsqueeze(1).broadcast_to([128, 2, L]))
                nc.gpsimd.tensor_add(out=FRpt[:, :], in0=FRpt[:, :],
                                     in1=tB[:, :])
                Fpt = FRpt[:, 0:L]
                Rpt = FRpt[:, L:2 * L]

                # (c) IN tile: [1,f0,f1,r0,r1,ff00,ff01,ff11,
                #               fr00,fr01,fr10,fr11] per q
                IN = ip.tile([128, Q * 12], F32, tag="IN")
                INv = IN.rearrange("p (q s) -> p q s", q=Q)
                nc.gpsimd.tensor_copy(out=INv[:, :, 0],
                                      in_=idF[:, 0:1].broadcast_to([128, Q]))
                nc.vector.tensor_copy(out=INv[:, :, 1:5], in_=nodecv)
                nc.vector.tensor_mul(out=INv[:, :, 5], in0=f0q, in1=f0q)
                nc.vector.tensor_mul(out=INv[:, :, 6], in0=f0q, in1=f1q)
                nc.vector.tensor_mul(out=INv[:, :, 7], in0=f1q, in1=f1q)
                nc.gpsimd.tensor_mul(
                    out=INv[:, :, 8:10],
                    in0=f0q.unsqueeze(2).broadcast_to([128, Q, 2]),
                    in1=rpair)
                nc.gpsimd.tensor_mul(
                    out=INv[:, :, 10:12],
                    in0=f1q.unsqueeze(2).broadcast_to([128, Q, 2]),
                    in1=rpair)

                # (d) 2x2 chain totals T = v @ W via PE transpose sandwich
                psT = psR.tile([96, 128], F32, tag="psT")
                nc.tensor.transpose(out=psT[:, :], in_=IN[:, :],
                                    identity=idn)
                b1t = sm.tile([96, 128], F32, tag="b1t")
                nc.vector.tensor_copy(out=b1t[:, :], in_=psT[:, :])
                psA = psR.tile([32, 128], F32, tag="psA")
                nc.tensor.matmul(out=psA[:, :],
                                 lhsT=csb[0:96, C_W:C_W + 32],
                                 rhs=b1t[:, :], start=True, stop=True)
                b2t = sm.tile([32, 128], F32, tag="b2t")
                nc.vector.tensor_copy(out=b2t[:, :], in_=psA[:, :])
                psT2 = psR.tile([128, 32], F32, tag="psT2")
                nc.tensor.transpose(out=psT2[:, :], in_=b2t[:, :],
                                    identity=idn[0:32, 0:32])
                Trow = lvb.tile([128, Q * 4], F32, tag="T")
                nc.vector.tensor_copy(out=Trow[:, :], in_=psT2[:, :])

                # (e) global Hillis-Steele: 7 partition passes with
                # cross-block wrap; combines read the PSUM directly
                Tcur = Trow
                for di, d in enumerate(SHIFT_DS):
                    pr = psS.tile([128, Q * 4], F32, tag="pr")
                    nc.tensor.matmul(out=pr[:, :],
                                     lhsT=shsb[:, di * 128:(di + 1) * 128],
                                     rhs=Tcur[:, :], start=True, stop=False)
                    nc.tensor.matmul(out=pr[:, 4:Q * 4],
                                     lhsT=shsb[:, (8 + di) * 128:
                                               (9 + di) * 128],
                                     rhs=Tcur[:, 0:(Q - 1) * 4],
                                     start=False, stop=False,
                                     skip_group_check=True)
                    nc.tensor.matmul(out=pr[:, 0:4],
                                     lhsT=csb[0:1, C_V + di * 128:
                                              C_V + (di + 1) * 128],
                                     rhs=idF[0:1, :],
                                     start=False, stop=True,
                                     skip_group_check=True)
                    Tn = lvb.tile([128, Q * 4], F32, tag="T")
                    TcV = Tcur.rearrange("p (q e) -> p q e", q=Q)
                    TnV = Tn.rearrange("p (q e) -> p q e", q=Q)
                    prV = pr.rearrange("p (q e) -> p q e", q=Q)
                    _c22(nc, sc2, TcV, prV, TnV, "e")
                    Tcur = Tn

                # (f) 3 free-dim block passes (shift by d blocks)
                for d in (1, 2, 4):
                    Tn = lvb.tile([128, Q * 4], F32, tag="T")
                    TcV = Tcur.rearrange("p (q e) -> p q e", q=Q)
                    TnV = Tn.rearrange("p (q e) -> p q e", q=Q)
                    nc.vector.tensor_copy(out=TnV[:, 0:d, :],
                                          in_=TcV[:, 0:d, :])
                    _c22(nc, sc2, TcV[:, d:Q, :], TcV[:, 0:Q - d, :],
                         TnV[:, d:Q, :], "f")
                    Tcur = Tn

                # (g) global exclusive prefix -> per-chain Phi, lift to x3
                prx = psR.tile([128, Q * 4], F32, tag="prx")
                nc.tensor.matmul(out=prx[:, :], lhsT=shsb[:, 0:128],
                                 rhs=Tcur[:, :], start=True, stop=False)
                nc.tensor.matmul(out=prx[:, 4:Q * 4],
                                 lhsT=shsb[:, 8 * 128:9 * 128],
                                 rhs=Tcur[:, 0:(Q - 1) * 4],
                                 start=False, stop=True,
                                 skip_group_check=True)
                Rexc = sm.tile([128, Q * 4], F32, tag="Rexc")
                RxV = Rexc.rearrange("p (q e) -> p q e", q=Q)
                prV = prx.rearrange("p (q e) -> p q e", q=Q)
                nc.vector.tensor_copy(out=RxV[:, 1:Q, :], in_=prV[:, 1:Q, :])
                nc.vector.tensor_add(out=RxV[:, 0, :], in0=prV[:, 0, :],
                                     in1=idZ)
                # own-block rows -> Phi [128, 4]
                tq = sm.tile([128, 4 * Q], F32, tag="tq")
                tqV = tq.rearrange("p (e q) -> p e q", e=4)
                nc.vector.tensor_mul(
                    out=tqV, in0=Rexc.rearrange("p (q e) -> p e q", q=Q),
                    in1=qmv.unsqueeze(1).broadcast_to([128, 4, Q]))
                Phi = sm.tile([128, 4], F32, tag="Phi")
                nc.vector.tensor_reduce(out=Phi[:, :], in_=tqV,
                                        axis=mybir.AxisListType.X,
                                        op=ALU.add)
                # lift: x3 = Sym2(Phi) @ (beta0, kappa0, nu0)
                P1 = sm.tile([128, 4], F32, tag="P1")
                P2 = sm.tile([128, 4], F32, tag="P2")
                P3 = sm.tile([128, 4], F32, tag="P3")
                a01 = Phi[:, 0:2]
                a23 = Phi[:, 2:4]
                nc.vector.tensor_mul(
                    out=P1.rearrange("p (a b) -> p a b", a=2),
                    in0=a01.unsqueeze(2).broadcast_to([128, 2, 2]),
                    in1=a01.unsqueeze(1).broadcast_to([128, 2, 2]))
                nc.vector.tensor_mul(
                    out=P2.rearrange("p (a b) -> p a b", a=2),
                    in0=a01.unsqueeze(2).broadcast_to([128, 2, 2]),
                    in1=a23.unsqueeze(1).broadcast_to([128, 2, 2]))
                nc.vector.tensor_mul(
                    out=P3.rearrange("p (a b) -> p a b", a=2),
                    in0=a23.unsqueeze(2).broadcast_to([128, 2, 2]),
                    in1=a23.unsqueeze(1).broadcast_to([128, 2, 2]))
                tw0 = sm.tile([128, 1], F32, tag="tw0")
                tw2 = sm.tile([128, 1], F32, tag="tw2")
                nc.vector.tensor_scalar(out=tw0[:, :], in0=s0v[:, 0:1],
                                        scalar1=2.0, scalar2=None,
                                        op0=ALU.mult)
                nc.vector.tensor_scalar(out=tw2[:, :], in0=s0v[:, 2:3],
                                        scalar1=2.0, scalar2=None,
                                        op0=ALU.mult)
                x3 = sm.tile([128, 3], F32, tag="x3")

                def _lift(dst, Pt, w0, w1a, w1b, w2):
                    nc.vector.tensor_scalar(out=dst, in0=Pt[:, 0:1],
                                            scalar1=w0, scalar2=None,
                                            op0=ALU.mult)
                    nc.vector.scalar_tensor_tensor(out=dst, in0=Pt[:, 1:2],
                                                   scalar=w1a, in1=dst,
                                                   op0=ALU.mult,
                                                   op1=ALU.add)
                    if w1b is not None:
                        nc.vector.scalar_tensor_tensor(out=dst,
                                                       in0=Pt[:, 2:3],
                                                       scalar=w1b, in1=dst,
                                                       op0=ALU.mult,
                                                       op1=ALU.add)
                    nc.vector.scalar_tensor_tensor(out=dst, in0=Pt[:, 3:4],
                                                   scalar=w2, in1=dst,
                                                   op0=ALU.mult,
                                                   op1=ALU.add)

                _lift(x3[:, 0:1], P1, s0v[:, 0:1], s0v[:, 1:2], None,
                      s0v[:, 2:3])
                _lift(x3[:, 1:2], P2, tw0[:, 0:1], s0v[:, 1:2],
                      s0v[:, 1:2], tw2[:, 0:1])
                _lift(x3[:, 2:3], P3, s0v[:, 0:1], s0v[:, 1:2], None,
                      s0v[:, 2:3])
                X0 = x3[:, 0:1]
                X1 = x3[:, 1:2]
                X2 = x3[:, 2:3]
                X3c = Phi[:, 0:1]     # alpha carry
                X4 = Phi[:, 2:3]      # lam carry

                # (h) states -> o7 strided columns
                out7 = stp.tile([CH, L * 7], F32, tag="out7")
                o7 = out7.rearrange("p (l c) -> p l c", c=7)
                p2x0 = sm.tile([CH, 1], F32, tag="p2x0")
                nx1 = sm.tile([CH, 1], F32, tag="nx1")
                n2x2 = sm.tile([CH, 1], F32, tag="n2x2")
                nx4 = sm.tile([CH, 1], F32, tag="nx4")
                nc.vector.tensor_scalar(out=p2x0[:, :], in0=X0, scalar1=2.0,
                                        scalar2=None, op0=ALU.mult)
                nc.vector.tensor_scalar(out=nx1[:, :], in0=X1, scalar1=-1.0,
                                        scalar2=None, op0=ALU.mult)
                nc.vector.tensor_scalar(out=n2x2[:, :], in0=X2, scalar1=-2.0,
                                        scalar2=None, op0=ALU.mult)
                nc.vector.tensor_scalar(out=nx4[:, :], in0=X4, scalar1=-1.0,
                                        scalar2=None, op0=ALU.mult)
                # DVE: beta, kappa, nu
                nc.vector.tensor_scalar(out=o7[:, :, 2], in0=Rpt,
                                        scalar1=nx1[:, 0:1], scalar2=X0,
                                        op0=ALU.mult, op1=ALU.add)
                tk = stp.tile([CH, L], F32, tag="tk")
                nc.vector.tensor_scalar(out=tk[:, :], in0=c1v,
                                        scalar1=p2x0[:, 0:1], scalar2=X1,
                                        op0=ALU.mult, op1=ALU.add)
                nc.vector.scalar_tensor_tensor(out=tk[:, :], in0=Fpt,
                                               scalar=nx1[:, 0:1],
                                               in1=tk[:, :], op0=ALU.mult,
                                               op1=ALU.add)
                nc.vector.scalar_tensor_tensor(out=o7[:, :, 3], in0=Rpt,
                                               scalar=n2x2[:, 0:1],
                                               in1=tk[:, :], op0=ALU.mult,
                                               op1=ALU.add)
                tn_ = stp.tile([CH, L], F32, tag="tn_")
                nc.vector.tensor_scalar(out=tn_[:, :], in0=c1v,
                                        scalar1=X1, scalar2=X2,
                                        op0=ALU.mult, op1=ALU.add)
                nc.vector.scalar_tensor_tensor(out=o7[:, :, 5], in0=Fpt,
                                               scalar=n2x2[:, 0:1],
                                               in1=tn_[:, :], op0=ALU.mult,
                                               op1=ALU.add)
                # Pool: alpha, lam
                tm_ = stp.tile([CH, L], F32, tag="tm_")
                tl_ = stp.tile([CH, L], F32, tag="tl_")
                nc.gpsimd.tensor_mul(out=tm_[:, :], in0=Rpt,
                                     in1=nx4[:, 0:1].broadcast_to([CH, L]))
                nc.gpsimd.tensor_add(out=o7[:, :, 0], in0=tm_[:, :],
                                     in1=X3c.broadcast_to([CH, L]))
                nc.gpsimd.tensor_mul(out=tl_[:, :], in0=c1v,
                                     in1=X3c.broadcast_to([CH, L]))
                nc.gpsimd.tensor_add(out=tl_[:, :], in0=tl_[:, :],
                                     in1=X4.broadcast_to([CH, L]))
                nc.gpsimd.tensor_mul(out=tm_[:, :], in0=Fpt,
                                     in1=nx4[:, 0:1].broadcast_to([CH, L]))
                nc.gpsimd.tensor_add(out=o7[:, :, 1], in0=tl_[:, :],
                                     in1=tm_[:, :])
                # kappa duplicate (cov is symmetric)
                nc.vector.tensor_copy(out=o7[:, :, 4], in_=o7[:, :, 3])

                # (i) log-SNR
                alp = o7[:, :, 0]
                lam = o7[:, :, 1]
                beta = o7[:, :, 2]
                kap = o7[:, :, 3]
                nu = o7[:, :, 5]
                ta = stp.tile([CH, L], F32, tag="ta")
                tb2 = stp.tile([CH, L], F32, tag="tb2")
                tcx = stp.tile([CH, L], F32, tag="tcx")
                td = stp.tile([CH, L], F32, tag="td")
                nc.vector.tensor_mul(out=ta[:, :], in0=lam, in1=lam)
                nc.vector.tensor_mul(out=ta[:, :], in0=beta, in1=ta[:, :])
                nc.vector.tensor_mul(out=tb2[:, :], in0=alp, in1=alp)
                nc.vector.tensor_mul(out=tb2[:, :], in0=nu, in1=tb2[:, :])
                nc.vector.tensor_add(out=ta[:, :], in0=ta[:, :],
                                     in1=tb2[:, :])
                nc.vector.tensor_mul(out=tb2[:, :], in0=alp, in1=lam)
                nc.vector.tensor_mul(out=tb2[:, :], in0=kap, in1=tb2[:, :])
                nc.vector.scalar_tensor_tensor(out=ta[:, :], in0=tb2[:, :],
                                               scalar=-2.0, in1=ta[:, :],
                                               op0=ALU.mult, op1=ALU.add)
                nc.gpsimd.tensor_mul(out=tcx[:, :], in0=kap, in1=kap)
                nc.gpsimd.tensor_mul(out=td[:, :], in0=beta, in1=nu)
                nc.gpsimd.tensor_sub(out=td[:, :], in0=td[:, :],
                                     in1=tcx[:, :])
                nc.scalar.activation(out=ta[:, :], in_=ta[:, :], func=AF.Ln,
                                     bias=0.0, scale=1.0)
                nc.scalar.activation(out=td[:, :], in_=td[:, :], func=AF.Ln,
                                     bias=0.0, scale=1.0)
                nc.vector.tensor_sub(out=o7[:, :, 6], in0=ta[:, :],
                                     in1=td[:, :])

                nc.sync.dma_start(out=out_d[:, :], in_=out7[:, :])
    if hoist:
        _hoist_matmul_waits(nc)
    return nc


_NC_CACHE = None
TRACE = False
LAST_EXEC_NS = None


def _w_matrix(dt):
    """2x2 chain-total entries as a linear map of the 12 node products
    [1,f0,f1,r0,r1,ff00,ff01,ff11,fr00,fr01,fr10,fr11]."""
    A, B, C = 3777475 / 784, 3751865 / 392, 3701035 / 784
    D, E = 1242085 / 784, 6261645 / 784
    d2 = dt * dt
    FLf = np.array([98.5, 97.5]) * dt
    IfDc = np.array([6402.5, 12707.5]) * d2
    IFc = np.array([12805.0, 6305.0]) * d2
    DL = L * dt
    W = np.zeros((12, 4), np.float64)
    # T2[0,0] = 1 - IrD
    W[0, 0] = 1.0
    W[3, 0], W[4, 0] = -IfDc
    # T2[0,1] = -RL + IrF
    W[3, 1], W[4, 1] = -FLf
    W[8:12, 1] = np.array([A, E, D, C]) * d2
    # T2[1,0] = DL - IfD
    W[0, 2] = DL
    W[1, 2], W[2, 2] = -IfDc
    # T2[1,1] = 1 - FL - IR + IfF
    W[0, 3] = 1.0
    W[1, 3], W[2, 3] = -FLf
    W[3, 3], W[4, 3] = -IFc
    W[5:8, 3] = np.array([A, B, C]) * d2
    return W


def kernel(**inputs):
    global _NC_CACHE, LAST_EXEC_NS
    t = np.asarray(inputs["t_range"], np.float32)
    t64 = t.astype(np.float64)
    dt = float((t64[-1] - t64[0]) / N)

    def f32(x):
        return np.ascontiguousarray(np.asarray(x, np.float32))

    w1cat = f32(inputs["fr_W1"])[:, 0]
    b1cat = f32(inputs["fr_b1"])
    w2t = f32(inputs["fr_W2"]).T
    b2cat = f32(inputs["fr_b2"])
    w3t = f32(inputs["fr_W3"]).T
    b3 = f32(inputs["fr_b3"])

    lbn = f32(inputs["log_beta_nu_zero"])
    beta0 = np.float32(np.exp(lbn[0]))
    nu0 = np.float32(np.exp(lbn[1]))
    rho0 = np.float32(1.0 / (1.0 + np.exp(-f32(inputs["log_rho_zero"])[0])))
    kappa0 = np.float32(rho0 * np.sqrt(beta0) * np.sqrt(nu0))

    wpack = np.zeros((128, 6), np.float32)
    wpack[:, 0] = b1cat[0:128]
    wpack[:, 1] = b1cat[128:256]
    wpack[:, 2] = b2cat[0:128]
    wpack[:, 3] = b2cat[128:256]
    wpack[:, 4] = w1cat[0:128]
    wpack[:, 5] = w1cat[128:256]
    w2p = np.zeros((128, 512), np.float32)
    w2p[:, 0:256] = w2t[0:128, :]
    w2p[:, 256:512] = w2t[128:256, :]
    w3p = np.zeros((128, 4), np.float32)
    w3p[:, 0:2] = w3t[0:128, :]
    w3p[:, 2:4] = w3t[128:256, :]

    # sample nodes, flipped chain<->partition map: chain g = q*128+(127-p)
    p_arr = np.arange(128)
    tsflat = np.zeros((1, SAMP), np.float32)
    for q in range(Q):
        for k in range(2):
            gi = q * 128 + (127 - p_arr)
            idxs = np.minimum(gi * L + L * k, N)
            tsflat[0, (q * 2 + k) * 128 + p_arr] = t[idxs]

    s_arr = np.arange(L, dtype=np.float64)
    id22 = np.array([1, 0, 0, 1], np.float32)
    cpack = np.zeros((128, CPW), np.float32)
    cpack[:, C_C1:C_C1 + L] = ((s_arr + 1.0) * dt)[None, :]
    cpack[:, C_C2:C_C2 + L] = (s_arr * (s_arr + 1.0) / (2.0 * L) * dt)[None, :]
    cpack[:, C_B3:C_B3 + 16] = np.tile(b3, 8)[None, :]
    cpack[127, C_IDZ:C_IDZ + 4] = id22
    cpack[:, C_S0:C_S0 + 3] = np.array([beta0, kappa0, nu0],
                                       np.float32)[None, :]
    cpack[:, C_IDF:C_IDF + 4] = id22[None, :]
    for di, d in enumerate(SHIFT_DS):
        cpack[0, C_V + di * 128 + (128 - d):C_V + (di + 1) * 128] = 1.0
    Wm = _w_matrix(dt).astype(np.float32)
    for q in range(Q):
        cpack[q * 12:(q + 1) * 12, C_W + q * 4:C_W + (q + 1) * 4] = Wm

    shifts = np.zeros((128, 15 * 128), np.float32)
    for di, d in enumerate(SHIFT_DS):
        shifts[:, di * 128:(di + 1) * 128] = np.eye(128, k=-d,
                                                    dtype=np.float32)
    shifts[:, 7 * 128:8 * 128] = np.eye(128, dtype=np.float32)
    for di, d in enumerate(SHIFT_DS):
        shifts[:, (8 + di) * 128:(9 + di) * 128] = np.eye(
            128, k=128 - d, dtype=np.float32)

    in_maps = []
    for c in range(NCORES):
        cpk = cpack.copy()
        cpk[:, C_QM + c] = 1.0
        in_maps.append({
            "tsflat": tsflat, "wpack": wpack, "w2p": w2p, "w3p": w3p,
            "cpack": cpk, "shifts": shifts,
        })

    if _NC_CACHE is None:
        _NC_CACHE = build_program()
    nc = _NC_CACHE
    res = run_bass_kernel_spmd(nc, in_maps, core_ids=list(range(NCORES)),
                               trace=TRACE)
    LAST_EXEC_NS = res.exec_time_ns

    full = np.empty((T, 7), np.float32)
    lsnr0 = np.float32(np.log(nu0) - np.log(beta0 * nu0 - kappa0 ** 2))
    full[0] = [1.0, 0.0, beta0, kappa0, kappa0, nu0, lsnr0]
    for c in range(NCORES):
        o = np.asarray(res.results[c]["out"], np.float32)
        o = o[::-1, :].reshape(PERC, 7)        # un-flip partitions
        lo = c * PERC
        hi = min(lo + PERC, N)
        full[lo + 1:hi + 1] = o[:hi - lo]
    return full
